# revision 13
# baseline (speedup 1.0000x reference)
"""Trainium2 Bass kernel for nn_CharRNN: 2-layer MI-GRU + large vocab projection.

Strategy (8 NeuronCores, SPMD, no collectives):
  - The sequential GRU recurrence (T=50 steps, B=100) is replicated on all
    8 cores: per-step matmul time is weight-column bound (independent of B),
    so batch-sharding would not speed it up, and replication avoids any
    cross-core synchronization.
  - The output projection logits = out @ softmax_w + b ([5000, 8000], 160 MB)
    is sharded over the vocab axis: core i computes columns [i*1000, (i+1)*1000)
    and writes its own 20 MB slice. The projection is interleaved into the
    recurrence loop (one 100-row stripe per timestep) so its matmuls fill the
    Tensor-engine stalls of the recurrent dependency chain and there is no
    serial tail.
  - Layer-0's input projection A0 = x@W0*alpha + beta1 depends only on the
    embedded inputs, so it is computed on the host and DMA-streamed per step,
    removing its matmuls + PSUM evacuations from the device entirely.

Layouts:
  - Gate/elementwise tensors: [B=100 partitions, features free], bf16 state;
    gate pre-activations kept f32 (the sigmoid/tanh argument is a small
    residual of values near 1 - rounding before the -1 bias is catastrophic).
  - Matmuls: out[B, N] = lhsT.T @ rhs with stationary lhsT = transposed
    activations [K=128 chunk, B] (bf16) and moving rhs = weight columns
    (bf16, 1 col/cycle; fp32 would be 2 cyc/col). Hidden-state transposes
    done on the PE via identity matmul; the r-path is split in 256-col
    halves so transposes/candidate-matmul start earlier (chain pipelining).
  - alpha/beta1/beta2/b are folded on the host:
      gate = sig((a*wx + b1) * (uh + b2/a) + (b - b1*b2/a))
"""

import os
import sys

sys.path.insert(0, "/opt/trn_rl_repo")

import ml_dtypes
import numpy as np

import concourse.bass as bass
import concourse.mybir as mybir
import concourse.tile as tile
from concourse.masks import make_identity

# ----------------------------------------------------------------------------
# Patch: the final SP Drain emitted by TileContext collects one semaphore wait
# per busy logical processor, but the walrus build in this container only
# lowers a limited number of sync-wait commands per CTRL instruction.  Split
# the waits across separate single-wait NoOps.
# ----------------------------------------------------------------------------
from concourse.vector_clock import ScopedClock
from bass_rust import SyncInfo

_MAXW = 1


def _patched_drain_and_barrier(self, tick_clock, wait_clock):
    nc = self.nc
    drain_inst = nc.sync.drain()
    wait_clock.add_sem_waits(
        drain_inst.ins, ScopedClock({None: tick_clock.global_clock})
    )
    si = drain_inst.ins.sync_info
    waits = list(si.on_wait) if si is not None else []
    if len(waits) > _MAXW:
        drain_inst.ins.sync_info = SyncInfo(
            on_wait=waits[:_MAXW], on_update=list(si.on_update)
        )
        for k in range(_MAXW, len(waits), _MAXW):
            nop = nc.sync.nop(nofuse=True)
            nop.ins.sync_info = SyncInfo(on_wait=waits[k : k + _MAXW], on_update=[])

    nc.all_engine_barrier()
    assert self.sems is not None
    popped = nc._tile_sem_poison_stack.pop()
    assert popped is self._sem_poison
    nc.clear_and_free_semaphores(list(self.sems.allocated().values()))
    nc.all_engine_barrier()


tile.TileContext._drain_and_barrier = _patched_drain_and_barrier

# ----------------------------------------------------------------------------
# Same walrus limitation applies to every engine instruction: split any
# instruction carrying more than _JLIM semaphore waits into preceding
# single-wait NoOps on the same engine (engines are in-order, so blocking on
# a prior NoOp is equivalent).  Done as a BIR-JSON post-pass on serialization.
# ----------------------------------------------------------------------------
import json as _json

_JLIM = 1
_orig_to_json_bytes = bass.Bass.to_json_bytes


def _split_waits_json(self) -> bytes:
    raw = _orig_to_json_bytes(self)
    d = _json.loads(raw)
    ctr = [0]

    def fix_block(blk):
        insts = blk.get("instructions")
        if insts:
            out = []
            for ins in insts:
                si = ins.get("sync_info")
                waits = (si or {}).get("on_wait") or []
                if len(waits) > _JLIM:
                    keep = waits[:_JLIM]
                    extra = waits[_JLIM:]
                    for k in range(0, len(extra), _JLIM):
                        ctr[0] += 1
                        out.append(
                            {
                                "debug": ins.get("debug", 0),
                                "engine": ins["engine"],
                                "ins": [],
                                "name": f"I-sw{ctr[0]}",
                                "opcode": "NoOp",
                                "outs": [],
                                "sync_info": {
                                    "on_wait": extra[k : k + _JLIM],
                                    "on_update": [],
                                },
                            }
                        )
                    si["on_wait"] = keep
                out.append(ins)
            blk["instructions"] = out
        for sub in blk.get("blocks", []) or []:
            fix_block(sub)

    for f in d.get("functions", []):
        for blk in f.get("blocks", []) or []:
            fix_block(blk)
    return _json.dumps(d).encode()


bass.Bass.to_json_bytes = _split_waits_json

# ----------------------------------------------------------------------------

B, T, H, E, V = 100, 50, 512, 128, 8000
G = 3 * H  # 1536
NCORES = 8
VS = V // NCORES  # 1000 vocab columns per core
KH = H // 128  # 4 K-chunks for H contraction
ROWS = B * T  # 5000 output rows
BF16 = mybir.dt.bfloat16
F32 = mybir.dt.float32
AF = mybir.ActivationFunctionType
ALU = mybir.AluOpType

# stash for test.py introspection
LAST_RESULTS = None


def _const_scalar(row, name):
    row = np.asarray(row, dtype=np.float64)
    lo, hi = row.min(), row.max()
    assert hi - lo < 1e-12, f"{name} is not a constant row; fast path invalid"
    return float(row[0])


def _bf16(a):
    return np.ascontiguousarray(np.asarray(a, dtype=np.float32)).astype(
        ml_dtypes.bfloat16
    )


def _fold_layer(W, U, b, alpha, beta1, beta2):
    """Host folding of the MI-GRU cell constants.

    gate_arg = alpha*wx*uh + beta1*uh + beta2*wx + b
             = (alpha*wx + beta1) * (uh + beta2/alpha) + (b - beta1*beta2/alpha)
    """
    W, U = np.asarray(W, np.float64), np.asarray(U, np.float64)
    alpha = np.asarray(alpha, np.float64)
    beta1 = np.asarray(beta1, np.float64)
    beta2 = np.asarray(beta2, np.float64)
    b = np.asarray(b, np.float64)
    Wf = W * alpha[None, :]
    r2 = beta2 / alpha
    d = b - beta1 * beta2 / alpha
    # per-range scalars (rows are constant in this problem)
    sc = {
        "b1g": _const_scalar(beta1[: 2 * H], "beta1_g"),
        "b1c": _const_scalar(beta1[2 * H :], "beta1_c"),
        "r2g": _const_scalar(r2[: 2 * H], "r2_g"),
        "r2c": _const_scalar(r2[2 * H :], "r2_c"),
        "dg": _const_scalar(d[: 2 * H], "d_g"),
        "dc": _const_scalar(d[2 * H :], "d_c"),
    }
    return Wf.astype(np.float32), np.asarray(U, np.float32), sc


def _build_program():
    nc = bass.Bass(
        "TRN2", target_bir_lowering=False, debug=False, num_devices=NCORES
    )

    # DRAM I/O (all recurrence weights bf16: fp32 moving operands stream at
    # 2 cycles/col on the PE, bf16 at 1)
    a0_d = nc.dram_tensor("a0", [T, B, G], F32, kind="ExternalInput").ap()
    u0_d = nc.dram_tensor("u0", [KH, 128, G], BF16, kind="ExternalInput").ap()
    w1f_d = nc.dram_tensor("w1f", [KH, 128, G], BF16, kind="ExternalInput").ap()
    u1_d = nc.dram_tensor("u1", [KH, 128, G], BF16, kind="ExternalInput").ap()
    wsm_d = nc.dram_tensor("wsm", [KH, 128, VS], BF16, kind="ExternalInput").ap()
    sbr_d = nc.dram_tensor("sbr", [128, VS], F32, kind="ExternalInput").ap()
    out_d = nc.dram_tensor("out", [ROWS, VS], F32, kind="ExternalOutput").ap()

    def build(tc, sc):
        nc = tc.nc
        cpool = tc.alloc_tile_pool(name="const", bufs=1)
        ld_engs = [nc.sync, nc.gpsimd, nc.scalar]
        # recurrence weights first: the first steps need them
        u0_s = cpool.tile([128, KH, G], BF16, tag="u0")
        w1f_s = cpool.tile([128, KH, G], BF16, tag="w1f")
        u1_s = cpool.tile([128, KH, G], BF16, tag="u1")
        for k in range(KH):
            ld_engs[k % 3].dma_start(u0_s[:, k, :], u0_d[k])
        for k in range(KH):
            ld_engs[(k + 1) % 3].dma_start(w1f_s[:, k, :], w1f_d[k])
            ld_engs[(k + 2) % 3].dma_start(u1_s[:, k, :], u1_d[k])
        wsm_s = cpool.tile([128, KH, VS], BF16, tag="wsm")
        for k in range(KH):
            ld_engs[k % 3].dma_start(wsm_s[:, k, :], wsm_d[k])
        sbr_s = cpool.tile([128, VS], F32, tag="sbr")
        nc.sync.dma_start(sbr_s[:], sbr_d[:])

        ident = cpool.tile([128, 128], BF16, tag="ident")
        make_identity(nc, ident[:])

        # bias constant tiles for ACT activations (bias must be an AP)
        _bias_tiles = {}

        def bias_ap(val, parts=B):
            val = float(val)
            if val not in _bias_tiles:
                bt = cpool.tile([128, 1], F32, tag=f"bias_{len(_bias_tiles)}")
                nc.vector.memset(bt[:], val)
                _bias_tiles[val] = bt
            return _bias_tiles[val][:parts]

        # initial states (zeros, bf16)
        h0_s = cpool.tile([B, H], BF16, tag="h0_init")
        h1_s = cpool.tile([B, H], BF16, tag="h1_init")
        h0T = cpool.tile([128, KH, B], BF16, tag="h0T_init")
        h1T = cpool.tile([128, KH, B], BF16, tag="h1T_init")
        nc.vector.memset(h0_s[:], 0.0)
        nc.vector.memset(h1_s[:], 0.0)
        nc.vector.memset(h0T[:], 0.0)
        nc.vector.memset(h1T[:], 0.0)

        # pools: PSUM banks: psA 1 + psG 4 + psT 2 + psP 1 = 8
        psA = tc.alloc_tile_pool(name="psA", bufs=1, space="PSUM")
        psG = tc.alloc_tile_pool(name="psG", bufs=4, space="PSUM")
        psT = tc.alloc_tile_pool(name="psT", bufs=2, space="PSUM")
        psP = tc.alloc_tile_pool(name="psP", bufs=1, space="PSUM")
        sb = tc.alloc_tile_pool(name="sb", bufs=2)

        # persistent bank of transposed layer-1 states: enables 128-row
        # projection M-tiles spanning step boundaries
        h1T_all = cpool.tile([128, KH, ROWS], BF16, tag="h1T_all")

        # PE warm-up: dummy transposes during the initial weight DMAs keep
        # the HAM clock-gate at 8/8 so step 0 starts at full clock
        wt = psT.tile([128, 128], BF16, tag="psT")
        for _ in range(40):
            nc.tensor.transpose(wt[:], ident[:], ident[:])

        def gate_pair(hT_fn, U_s):
            """Emit the r/z gate matmul groups (ready at cell start)."""
            psr = psG.tile([B, 512], F32, tag="psG")
            for k in range(KH):
                nc.tensor.matmul(
                    psr[:], hT_fn(k), U_s[:, k, 0:512],
                    start=(k == 0), stop=(k == KH - 1),
                )
            psz = psG.tile([B, 512], F32, tag="psG")
            last = None
            for k in range(KH):
                last = nc.tensor.matmul(
                    psz[:], hT_fn(k), U_s[:, k, 512:1024],
                    start=(k == 0), stop=(k == KH - 1),
                )
            return psr, psz, last

        def cell_rest(lt, psr, psz, A_ap, h_prev, U_s, sc_l, hT_evac):
            """Everything after the r/z matmuls. hT_evac(src_ap) stores the
            transposed new state. Returns new_h [B,H] bf16."""
            # --- r path ---
            m_r = sb.tile([B, 512], F32, tag=f"mr{lt}", bufs=2)
            nc.vector.scalar_tensor_tensor(
                m_r[:], psr[:], sc_l["r2g"], A_ap[:, :512], ALU.add, ALU.mult
            )
            r = sb.tile([B, 512], BF16, tag=f"r{lt}", bufs=2)
            nc.scalar.activation(r[:], m_r[:], AF.Sigmoid, bias=bias_ap(sc_l["dg"]))
            rh = sb.tile([B, 512], BF16, tag=f"rh{lt}", bufs=2)
            nc.vector.tensor_mul(rh[:], r[:], h_prev[:])
            pst_r = psT.tile([128, KH, B], BF16, tag="psT")
            for j in range(KH):
                nc.tensor.transpose(
                    pst_r[:, j, :], rh[:, j * 128 : (j + 1) * 128], ident[:B, :B]
                )
            rhT = sb.tile([128, KH, B], BF16, tag=f"rhT{lt}", bufs=2)
            nc.scalar.activation(rhT[:, :, :], pst_r[:, :, :], AF.Copy)

            # --- candidate matmul ---
            psc = psG.tile([B, 512], F32, tag="psG")
            for k in range(KH):
                nc.tensor.matmul(
                    psc[:], rhT[:, k, :], U_s[:, k, 1024:1536],
                    start=(k == 0), stop=(k == KH - 1),
                )

            # --- z gate (early: q needs z at the tail) ---
            m_z = sb.tile([B, 512], F32, tag=f"mz{lt}", bufs=2)
            nc.vector.scalar_tensor_tensor(
                m_z[:], psz[:], sc_l["r2g"], A_ap[:, 512:1024], ALU.add, ALU.mult
            )
            z = sb.tile([B, 512], BF16, tag=f"z{lt}", bufs=2)
            nc.scalar.activation(z[:], m_z[:], AF.Sigmoid, bias=bias_ap(sc_l["dg"]))
            zh = sb.tile([B, 512], BF16, tag=f"zh{lt}", bufs=2)
            nc.vector.tensor_mul(zh[:], z[:], h_prev[:])

            # --- c path + combine: new_h = zh - (z-1)*c ---
            m_c = sb.tile([B, 512], F32, tag=f"mc{lt}", bufs=2)
            nc.vector.scalar_tensor_tensor(
                m_c[:], psc[:], sc_l["r2c"], A_ap[:, 1024:], ALU.add, ALU.mult
            )
            cc = sb.tile([B, 512], BF16, tag=f"c{lt}", bufs=2)
            nc.scalar.activation(cc[:], m_c[:], AF.Tanh, bias=bias_ap(sc_l["dc"]))
            q = sb.tile([B, 512], BF16, tag=f"q{lt}", bufs=2)
            nc.vector.scalar_tensor_tensor(
                q[:], z[:], 1.0, cc[:], ALU.subtract, ALU.mult
            )
            nh = sb.tile([B, H], BF16, tag=f"h{lt}", bufs=2)
            nc.vector.tensor_tensor(nh[:], zh[:], q[:], ALU.subtract)
            pst_h = psT.tile([128, KH, B], BF16, tag="psT")
            for j in range(KH):
                nc.tensor.transpose(
                    pst_h[:, j, :], nh[:, j * 128 : (j + 1) * 128], ident[:B, :B]
                )
            hT_evac(pst_h[:, :, :])
            return nh

        dma_engines = [nc.sync, nc.gpsimd, nc.scalar]

        def emit_proj_bank(m, nb, lo, dep_ins):
            mrows = min(128, ROWS - m * 128)
            ns = slice(nb * 500, (nb + 1) * 500)
            psp = psP.tile([128, 500], F32, tag="psP")
            for k in range(KH):
                mm = nc.tensor.matmul(
                    psp[:mrows, :],
                    h1T_all[:, k, m * 128 : m * 128 + mrows],
                    wsm_s[:, k, ns],
                    start=(k == 0), stop=(k == KH - 1),
                )
                if k == 0 and dep_ins is not None:
                    tile.add_dep_helper(
                        mm.ins, dep_ins, reason="delay proj into chain window"
                    )
            nc.vector.tensor_add(
                lo[:mrows, ns], psp[:mrows, :], sbr_s[:mrows, ns]
            )

        def finish_proj(m, lo):
            mrows = min(128, ROWS - m * 128)
            dma_engines[m % 3].dma_start(
                out_d[m * 128 : m * 128 + mrows, :], lo[:mrows, :]
            )

        # projection M-tile emission schedule (cell1 lags one slot):
        # h1T[t] is produced in slot t+1, so tile m is emittable in slot
        # (last_row//B) + 2
        proj_emit = {}
        tail_tiles = []
        for m in range((ROWS + 127) // 128):
            last_row = min((m + 1) * 128, ROWS) - 1
            t_emit = last_row // B + 2
            if t_emit <= T:
                proj_emit.setdefault(t_emit, []).append(m)
            else:
                tail_tiles.append(m)

        sc0, sc1 = sc["l0"], sc["l1"]
        h0T_fn = lambda k: h0T[:, k, :]
        h1T_fn = lambda k: h1T[:, k, :]
        A1_prev = None
        # Slot s runs cell0[s] and, fully independently, cell1[s-1]
        # (whose A1 input was computed last slot as gap filler).
        for s in range(T + 1):
            t1 = s - 1  # cell1's timestep this slot

            # ---- cell 1 for step t1: all inputs ready at slot start ----
            if t1 >= 0:
                psr1, psz1, _ = gate_pair(h1T_fn, u1_s)
                h1_s = cell_rest(
                    "l1", psr1, psz1, A1_prev[:], h1_s, u1_s, sc1,
                    (lambda t_: lambda src: nc.scalar.activation(
                        h1T_all[:, :, t_ * B : (t_ + 1) * B], src, AF.Copy,
                    ))(t1),
                )
                h1T_fn = (lambda t_: lambda k: h1T_all[:, k, t_ * B : (t_ + 1) * B])(t1)

            # ---- projection tiles due this slot (pure filler) ----
            for m in proj_emit.get(s, []):
                lo = sb.tile([128, VS], F32, tag="lout", bufs=2)
                emit_proj_bank(m, 0, lo, None)
                emit_proj_bank(m, 1, lo, None)
                finish_proj(m, lo)

            # ---- cell 0 for step s ----
            if s < T:
                a0_sb = sb.tile([B, G], F32, tag="A0", bufs=3)
                dma_engines[s % 3].dma_start(a0_sb[:], a0_d[s])
                psr0, psz0, _ = gate_pair(h0T_fn, u0_s)
                h0T_new = sb.tile([128, KH, B], BF16, tag="hTl0", bufs=2)
                h0_s = cell_rest(
                    "l0", psr0, psz0, a0_sb[:], h0_s, u0_s, sc0,
                    lambda src: nc.scalar.activation(h0T_new[:, :, :], src, AF.Copy),
                )
                h0T_fn = (lambda tl: lambda k: tl[:, k, :])(h0T_new)

                # ---- A1[s] = h0[s] @ W1f (+b1): filler for this slot ----
                A1_s = sb.tile([B, G], F32, tag="A1", bufs=2)
                for n in range(3):
                    ns = slice(n * 512, (n + 1) * 512)
                    psa = psA.tile([B, 512], F32, tag="psA")
                    for ki in range(KH):
                        nc.tensor.matmul(
                            psa[:], h0T_fn(ki), w1f_s[:, ki, ns],
                            start=(ki == 0), stop=(ki == KH - 1),
                        )
                    if n == 1:
                        nc.vector.tensor_scalar_add(A1_s[:, ns], psa[:], sc1["b1g"])
                    else:
                        nc.scalar.activation(
                            A1_s[:, ns], psa[:], AF.Identity,
                            bias=bias_ap(sc1["b1g"] if n == 0 else sc1["b1c"]),
                        )
                A1_prev = A1_s

        for m in tail_tiles:
            lo = sb.tile([128, VS], F32, tag="lout", bufs=2)
            emit_proj_bank(m, 0, lo, None)
            emit_proj_bank(m, 1, lo, None)
            finish_proj(m, lo)

        for p in (sb, psP, psT, psG, psA, cpool):
            p.release()

    return nc, build


def kernel(**inputs):
    global LAST_RESULTS
    inp = {k: np.asarray(v) for k, v in inputs.items()}

    # ---- host prep ----
    xs = np.asarray(inp["embedding"], np.float32)[np.asarray(inp["input_data"])]
    # xs: [B, T, E]

    W0f, U0, sc0 = _fold_layer(
        inp["W0"], inp["U0"], inp["b0"], inp["alpha0"], inp["beta1_0"], inp["beta2_0"]
    )
    W1f, U1, sc1 = _fold_layer(
        inp["W1"], inp["U1"], inp["b1"], inp["alpha1"], inp["beta1_1"], inp["beta2_1"]
    )
    for sc in (sc0, sc1):
        assert abs(sc["b1g"] - sc["b1c"]) < 1e-12, "A bias fold needs b1g == b1c"

    # layer-0 input projection on host: A0[t] = x[t] @ W0f + b1  ([T, B, G] f32)
    A0 = np.einsum("bte,eg->tbg", xs, W0f.astype(np.float32)) + np.float32(
        sc0["b1g"]
    )
    A0 = np.ascontiguousarray(A0, dtype=np.float32)

    u0c = np.ascontiguousarray(U0.reshape(KH, 128, G))
    w1c = np.ascontiguousarray(W1f.reshape(KH, 128, G))
    u1c = np.ascontiguousarray(U1.reshape(KH, 128, G))

    wsm = np.asarray(inp["softmax_w"], np.float32)  # [H, V]
    sb = np.asarray(inp["softmax_b"], np.float32)  # [V]

    nc, build = _build_program()
    with tile.TileContext(nc) as tc:
        build(tc, {"l0": sc0, "l1": sc1})

    base_map = {
        "a0": A0,
        "u0": _bf16(u0c),
        "w1f": _bf16(w1c),
        "u1": _bf16(u1c),
    }
    in_maps = []
    for c in range(NCORES):
        vs = slice(c * VS, (c + 1) * VS)
        m = dict(base_map)
        m["wsm"] = _bf16(np.ascontiguousarray(wsm[:, vs]).reshape(KH, 128, VS))
        m["sbr"] = np.ascontiguousarray(
            np.tile(sb[vs][None, :], (128, 1)).astype(np.float32)
        )
        in_maps.append(m)

    from concourse.bass_utils import run_bass_kernel_spmd

    trace = bool(int(os.environ.get("KERNEL_TRACE", "0")))
    res = run_bass_kernel_spmd(
        nc, in_maps, core_ids=list(range(NCORES)), trace=trace
    )
    LAST_RESULTS = res

    # ---- assemble: concat vocab slices, reorder rows (t-major -> b-major) ----
    logits_tb = np.concatenate(
        [res.results[c]["out"] for c in range(NCORES)], axis=1
    )  # [T*B, V]
    logits = (
        logits_tb.reshape(T, B, V).transpose(1, 0, 2).reshape(B * T, V)
    )
    return np.ascontiguousarray(logits.astype(np.float32))


# revision 14
# speedup vs baseline: 1.1505x; 1.1505x over previous
"""Trainium2 Bass kernel for nn_CharRNN: 2-layer MI-GRU + large vocab projection.

Strategy (8 NeuronCores, SPMD, no collectives):
  - The sequential GRU recurrence (T=50 steps, B=100) is replicated on all
    8 cores: per-step matmul time is weight-column bound (independent of B),
    so batch-sharding would not speed it up, and replication avoids any
    cross-core synchronization.
  - The output projection logits = out @ softmax_w + b ([5000, 8000], 160 MB)
    is sharded over the vocab axis: core i computes columns [i*1000, (i+1)*1000)
    and writes its own 20 MB slice. The projection is interleaved into the
    recurrence loop (one 100-row stripe per timestep) so its matmuls fill the
    Tensor-engine stalls of the recurrent dependency chain and there is no
    serial tail.
  - Layer-0's input projection A0 = x@W0*alpha + beta1 depends only on the
    embedded inputs, so it is computed on the host and DMA-streamed per step,
    removing its matmuls + PSUM evacuations from the device entirely.

Layouts:
  - Gate/elementwise tensors: [B=100 partitions, features free], bf16 state;
    gate pre-activations kept f32 (the sigmoid/tanh argument is a small
    residual of values near 1 - rounding before the -1 bias is catastrophic).
  - Matmuls: out[B, N] = lhsT.T @ rhs with stationary lhsT = transposed
    activations [K=128 chunk, B] (bf16) and moving rhs = weight columns
    (bf16, 1 col/cycle; fp32 would be 2 cyc/col). Hidden-state transposes
    done on the PE via identity matmul; the r-path is split in 256-col
    halves so transposes/candidate-matmul start earlier (chain pipelining).
  - alpha/beta1/beta2/b are folded on the host:
      gate = sig((a*wx + b1) * (uh + b2/a) + (b - b1*b2/a))
"""

import os
import sys

sys.path.insert(0, "/opt/trn_rl_repo")

import ml_dtypes
import numpy as np

import concourse.bass as bass
import concourse.mybir as mybir
import concourse.tile as tile
from concourse.masks import make_identity

# ----------------------------------------------------------------------------
# Patch: the final SP Drain emitted by TileContext collects one semaphore wait
# per busy logical processor, but the walrus build in this container only
# lowers a limited number of sync-wait commands per CTRL instruction.  Split
# the waits across separate single-wait NoOps.
# ----------------------------------------------------------------------------
from concourse.vector_clock import ScopedClock
from bass_rust import SyncInfo

_MAXW = 1


def _patched_drain_and_barrier(self, tick_clock, wait_clock):
    nc = self.nc
    drain_inst = nc.sync.drain()
    wait_clock.add_sem_waits(
        drain_inst.ins, ScopedClock({None: tick_clock.global_clock})
    )
    si = drain_inst.ins.sync_info
    waits = list(si.on_wait) if si is not None else []
    if len(waits) > _MAXW:
        drain_inst.ins.sync_info = SyncInfo(
            on_wait=waits[:_MAXW], on_update=list(si.on_update)
        )
        for k in range(_MAXW, len(waits), _MAXW):
            nop = nc.sync.nop(nofuse=True)
            nop.ins.sync_info = SyncInfo(on_wait=waits[k : k + _MAXW], on_update=[])

    nc.all_engine_barrier()
    assert self.sems is not None
    popped = nc._tile_sem_poison_stack.pop()
    assert popped is self._sem_poison
    nc.clear_and_free_semaphores(list(self.sems.allocated().values()))
    nc.all_engine_barrier()


tile.TileContext._drain_and_barrier = _patched_drain_and_barrier

# ----------------------------------------------------------------------------
# Same walrus limitation applies to every engine instruction: split any
# instruction carrying more than _JLIM semaphore waits into preceding
# single-wait NoOps on the same engine (engines are in-order, so blocking on
# a prior NoOp is equivalent).  Done as a BIR-JSON post-pass on serialization.
# ----------------------------------------------------------------------------
import json as _json

_JLIM = 1
_orig_to_json_bytes = bass.Bass.to_json_bytes


def _split_waits_json(self) -> bytes:
    raw = _orig_to_json_bytes(self)
    d = _json.loads(raw)
    ctr = [0]

    def fix_block(blk):
        insts = blk.get("instructions")
        if insts:
            out = []
            for ins in insts:
                si = ins.get("sync_info")
                waits = (si or {}).get("on_wait") or []
                if len(waits) > _JLIM:
                    keep = waits[:_JLIM]
                    extra = waits[_JLIM:]
                    for k in range(0, len(extra), _JLIM):
                        ctr[0] += 1
                        out.append(
                            {
                                "debug": ins.get("debug", 0),
                                "engine": ins["engine"],
                                "ins": [],
                                "name": f"I-sw{ctr[0]}",
                                "opcode": "NoOp",
                                "outs": [],
                                "sync_info": {
                                    "on_wait": extra[k : k + _JLIM],
                                    "on_update": [],
                                },
                            }
                        )
                    si["on_wait"] = keep
                out.append(ins)
            blk["instructions"] = out
        for sub in blk.get("blocks", []) or []:
            fix_block(sub)

    for f in d.get("functions", []):
        for blk in f.get("blocks", []) or []:
            fix_block(blk)
    return _json.dumps(d).encode()


bass.Bass.to_json_bytes = _split_waits_json

# ----------------------------------------------------------------------------

B, T, H, E, V = 100, 50, 512, 128, 8000
G = 3 * H  # 1536
NCORES = 8
VS = V // NCORES  # 1000 vocab columns per core
KH = H // 128  # 4 K-chunks for H contraction
ROWS = B * T  # 5000 output rows
BF16 = mybir.dt.bfloat16
F32 = mybir.dt.float32
AF = mybir.ActivationFunctionType
ALU = mybir.AluOpType

# stash for test.py introspection
LAST_RESULTS = None


def _const_scalar(row, name):
    row = np.asarray(row, dtype=np.float64)
    lo, hi = row.min(), row.max()
    assert hi - lo < 1e-12, f"{name} is not a constant row; fast path invalid"
    return float(row[0])


def _bf16(a):
    return np.ascontiguousarray(np.asarray(a, dtype=np.float32)).astype(
        ml_dtypes.bfloat16
    )


def _fold_layer(W, U, b, alpha, beta1, beta2):
    """Host folding of the MI-GRU cell constants.

    gate_arg = alpha*wx*uh + beta1*uh + beta2*wx + b
             = (alpha*wx + beta1) * (uh + beta2/alpha) + (b - beta1*beta2/alpha)
    """
    W, U = np.asarray(W, np.float64), np.asarray(U, np.float64)
    alpha = np.asarray(alpha, np.float64)
    beta1 = np.asarray(beta1, np.float64)
    beta2 = np.asarray(beta2, np.float64)
    b = np.asarray(b, np.float64)
    Wf = W * alpha[None, :]
    r2 = beta2 / alpha
    d = b - beta1 * beta2 / alpha
    # per-range scalars (rows are constant in this problem)
    sc = {
        "b1g": _const_scalar(beta1[: 2 * H], "beta1_g"),
        "b1c": _const_scalar(beta1[2 * H :], "beta1_c"),
        "r2g": _const_scalar(r2[: 2 * H], "r2_g"),
        "r2c": _const_scalar(r2[2 * H :], "r2_c"),
        "dg": _const_scalar(d[: 2 * H], "d_g"),
        "dc": _const_scalar(d[2 * H :], "d_c"),
    }
    return Wf.astype(np.float32), np.asarray(U, np.float32), sc


def _build_program():
    nc = bass.Bass(
        "TRN2", target_bir_lowering=False, debug=False, num_devices=NCORES
    )

    # DRAM I/O (all recurrence weights bf16: fp32 moving operands stream at
    # 2 cycles/col on the PE, bf16 at 1)
    a0_d = nc.dram_tensor("a0", [T, B, G], F32, kind="ExternalInput").ap()
    u0_d = nc.dram_tensor("u0", [KH, 128, G], BF16, kind="ExternalInput").ap()
    w1f_d = nc.dram_tensor("w1f", [KH, 128, G], BF16, kind="ExternalInput").ap()
    u1_d = nc.dram_tensor("u1", [KH, 128, G], BF16, kind="ExternalInput").ap()
    wsm_d = nc.dram_tensor("wsm", [KH, 128, VS], BF16, kind="ExternalInput").ap()
    sbr_d = nc.dram_tensor("sbr", [128, VS], F32, kind="ExternalInput").ap()
    out_d = nc.dram_tensor("out", [ROWS, VS], F32, kind="ExternalOutput").ap()

    def build(tc, sc):
        nc = tc.nc
        cpool = tc.alloc_tile_pool(name="const", bufs=1)
        ld_engs = [nc.sync, nc.gpsimd, nc.scalar]
        # recurrence weights first: the first steps need them
        u0_s = cpool.tile([128, KH, G], BF16, tag="u0")
        w1f_s = cpool.tile([128, KH, G], BF16, tag="w1f")
        u1_s = cpool.tile([128, KH, G], BF16, tag="u1")
        for k in range(KH):
            ld_engs[k % 3].dma_start(u0_s[:, k, :], u0_d[k])
        for k in range(KH):
            ld_engs[(k + 1) % 3].dma_start(w1f_s[:, k, :], w1f_d[k])
            ld_engs[(k + 2) % 3].dma_start(u1_s[:, k, :], u1_d[k])
        wsm_s = cpool.tile([128, KH, VS], BF16, tag="wsm")
        for k in range(KH):
            ld_engs[k % 3].dma_start(wsm_s[:, k, :], wsm_d[k])
        sbr_s = cpool.tile([128, VS], F32, tag="sbr")
        nc.sync.dma_start(sbr_s[:], sbr_d[:])

        ident = cpool.tile([128, 128], BF16, tag="ident")
        make_identity(nc, ident[:])

        # bias constant tiles for ACT activations (bias must be an AP)
        _bias_tiles = {}

        def bias_ap(val, parts=B):
            val = float(val)
            if val not in _bias_tiles:
                bt = cpool.tile([128, 1], F32, tag=f"bias_{len(_bias_tiles)}")
                nc.vector.memset(bt[:], val)
                _bias_tiles[val] = bt
            return _bias_tiles[val][:parts]

        # initial states (zeros, bf16)
        h0_s = cpool.tile([B, H], BF16, tag="h0_init")
        h1_s = cpool.tile([B, H], BF16, tag="h1_init")
        h0T = cpool.tile([128, KH, B], BF16, tag="h0T_init")
        h1T = cpool.tile([128, KH, B], BF16, tag="h1T_init")
        nc.vector.memset(h0_s[:], 0.0)
        nc.vector.memset(h1_s[:], 0.0)
        nc.vector.memset(h0T[:], 0.0)
        nc.vector.memset(h1T[:], 0.0)

        # pools: PSUM banks: psA 1 + psG 5 (gates + projection) + psT 2 = 8
        psA = tc.alloc_tile_pool(name="psA", bufs=1, space="PSUM")
        psG = tc.alloc_tile_pool(name="psG", bufs=5, space="PSUM")
        psT = tc.alloc_tile_pool(name="psT", bufs=2, space="PSUM")
        sb = tc.alloc_tile_pool(name="sb", bufs=2)

        # persistent bank of transposed layer-1 states: enables 128-row
        # projection M-tiles spanning step boundaries
        h1T_all = cpool.tile([128, KH, ROWS], BF16, tag="h1T_all")

        # PE warm-up: dummy transposes during the initial weight DMAs keep
        # the HAM clock-gate at 8/8 so step 0 starts at full clock
        wt = psT.tile([128, 128], BF16, tag="psT")
        for _ in range(40):
            nc.tensor.transpose(wt[:], ident[:], ident[:])

        def gate_pair(hT_fn, U_s):
            """Emit the r/z gate matmul groups (ready at cell start)."""
            psr = psG.tile([B, 512], F32, tag="psG")
            for k in range(KH):
                nc.tensor.matmul(
                    psr[:], hT_fn(k), U_s[:, k, 0:512],
                    start=(k == 0), stop=(k == KH - 1),
                )
            psz = psG.tile([B, 512], F32, tag="psG")
            last = None
            for k in range(KH):
                last = nc.tensor.matmul(
                    psz[:], hT_fn(k), U_s[:, k, 512:1024],
                    start=(k == 0), stop=(k == KH - 1),
                )
            return psr, psz, last

        def cell_rest(lt, psr, psz, A_ap, h_prev, U_s, sc_l, hT_evac):
            """Everything after the r/z matmuls. hT_evac(hf, src_ap) stores
            the transposed new state's half hf. Returns new_h [B,H] bf16."""
            # --- r path, split in 256-col halves for chain pipelining ---
            rh = sb.tile([B, 512], BF16, tag=f"rh{lt}", bufs=2)
            pst_r = psT.tile([128, KH, B], BF16, tag="psT")
            rhT = sb.tile([128, KH, B], BF16, tag=f"rhT{lt}", bufs=2)
            for hf in range(2):
                hs = slice(hf * 256, (hf + 1) * 256)
                m_r = sb.tile([B, 256], F32, tag=f"mr{lt}{hf}", bufs=2)
                nc.vector.scalar_tensor_tensor(
                    m_r[:], psr[:, hs], sc_l["r2g"], A_ap[:, hs], ALU.add, ALU.mult
                )
                r = sb.tile([B, 256], BF16, tag=f"r{lt}{hf}", bufs=2)
                nc.scalar.activation(
                    r[:], m_r[:], AF.Sigmoid, bias=bias_ap(sc_l["dg"])
                )
                nc.vector.tensor_mul(rh[:, hs], r[:], h_prev[:, hs])
                for jj in range(2):
                    j = hf * 2 + jj
                    nc.tensor.transpose(
                        pst_r[:, j, :], rh[:, j * 128 : (j + 1) * 128],
                        ident[:B, :B],
                    )
                nc.scalar.activation(
                    rhT[:, hf * 2 : hf * 2 + 2, :],
                    pst_r[:, hf * 2 : hf * 2 + 2, :],
                    AF.Copy,
                )

            # --- candidate: psc accumulates over rhT chunks ---
            psc = psG.tile([B, 512], F32, tag="psG")
            for k in range(KH):
                nc.tensor.matmul(
                    psc[:], rhT[:, k, :], U_s[:, k, 1024:1536],
                    start=(k == 0), stop=(k == KH - 1),
                )

            # --- z gate elementwise (early: q needs z at the tail) ---
            m_z = sb.tile([B, 512], F32, tag=f"mz{lt}", bufs=2)
            nc.vector.scalar_tensor_tensor(
                m_z[:], psz[:], sc_l["r2g"], A_ap[:, 512:1024], ALU.add, ALU.mult
            )
            z = sb.tile([B, 512], BF16, tag=f"z{lt}", bufs=2)
            nc.scalar.activation(z[:], m_z[:], AF.Sigmoid, bias=bias_ap(sc_l["dg"]))
            zh = sb.tile([B, 512], BF16, tag=f"zh{lt}", bufs=2)
            nc.vector.tensor_mul(zh[:], z[:], h_prev[:])

            # --- c path + combine, split in halves: new_h = zh - (z-1)*c ---
            nh = sb.tile([B, H], BF16, tag=f"h{lt}", bufs=2)
            pst_h = psT.tile([128, KH, B], BF16, tag="psT")
            for hf in range(2):
                hs = slice(hf * 256, (hf + 1) * 256)
                m_c = sb.tile([B, 256], F32, tag=f"mc{lt}{hf}", bufs=2)
                nc.vector.scalar_tensor_tensor(
                    m_c[:], psc[:, hs], sc_l["r2c"],
                    A_ap[:, 1024 + hf * 256 : 1024 + (hf + 1) * 256],
                    ALU.add, ALU.mult,
                )
                cc = sb.tile([B, 256], BF16, tag=f"c{lt}{hf}", bufs=2)
                nc.scalar.activation(
                    cc[:], m_c[:], AF.Tanh, bias=bias_ap(sc_l["dc"])
                )
                q = sb.tile([B, 256], BF16, tag=f"q{lt}{hf}", bufs=2)
                nc.vector.scalar_tensor_tensor(
                    q[:], z[:, hs], 1.0, cc[:], ALU.subtract, ALU.mult
                )
                nc.vector.tensor_tensor(nh[:, hs], zh[:, hs], q[:], ALU.subtract)
                for jj in range(2):
                    j = hf * 2 + jj
                    nc.tensor.transpose(
                        pst_h[:, j, :], nh[:, j * 128 : (j + 1) * 128],
                        ident[:B, :B],
                    )
                hT_evac(hf, pst_h[:, hf * 2 : hf * 2 + 2, :])
            return nh

        dma_engines = [nc.sync, nc.gpsimd, nc.scalar]

        def emit_proj_bank(m, nb, lo, dep_ins):
            mrows = min(128, ROWS - m * 128)
            ns = slice(nb * 500, (nb + 1) * 500)
            psp = psG.tile([128, 512], F32, tag="psG")
            for k in range(KH):
                mm = nc.tensor.matmul(
                    psp[:mrows, :500],
                    h1T_all[:, k, m * 128 : m * 128 + mrows],
                    wsm_s[:, k, ns],
                    start=(k == 0), stop=(k == KH - 1),
                )
                if k == 0 and dep_ins is not None:
                    tile.add_dep_helper(
                        mm.ins, dep_ins, reason="delay proj into chain window"
                    )
            nc.vector.tensor_add(
                lo[:mrows, ns], psp[:mrows, :500], sbr_s[:mrows, ns]
            )

        def finish_proj(m, lo):
            mrows = min(128, ROWS - m * 128)
            dma_engines[m % 3].dma_start(
                out_d[m * 128 : m * 128 + mrows, :], lo[:mrows, :]
            )

        # projection M-tile emission schedule (cell1 lags one slot):
        # h1T[t] is produced in slot t+1, so tile m is emittable in slot
        # (last_row//B) + 2
        proj_emit = {}
        tail_tiles = []
        for m in range((ROWS + 127) // 128):
            last_row = min((m + 1) * 128, ROWS) - 1
            t_emit = last_row // B + 2
            if t_emit <= T:
                proj_emit.setdefault(t_emit, []).append(m)
            else:
                tail_tiles.append(m)

        sc0, sc1 = sc["l0"], sc["l1"]
        h0T_fn = lambda k: h0T[:, k, :]
        h1T_fn = lambda k: h1T[:, k, :]
        A1_prev = None
        # Slot s runs cell0[s] and, fully independently, cell1[s-1]
        # (whose A1 input was computed last slot as gap filler).
        for s in range(T + 1):
            t1 = s - 1  # cell1's timestep this slot

            # ---- cell 1 for step t1: all inputs ready at slot start ----
            if t1 >= 0:
                psr1, psz1, _ = gate_pair(h1T_fn, u1_s)
                h1_s = cell_rest(
                    "l1", psr1, psz1, A1_prev[:], h1_s, u1_s, sc1,
                    (lambda t_: lambda hf, src: nc.scalar.activation(
                        h1T_all[:, hf * 2 : hf * 2 + 2, t_ * B : (t_ + 1) * B],
                        src, AF.Copy,
                    ))(t1),
                )
                h1T_fn = (lambda t_: lambda k: h1T_all[:, k, t_ * B : (t_ + 1) * B])(t1)

            # ---- projection tiles due this slot (pure filler) ----
            for m in proj_emit.get(s, []):
                lo = sb.tile([128, VS], F32, tag="lout", bufs=2)
                emit_proj_bank(m, 0, lo, None)
                emit_proj_bank(m, 1, lo, None)
                finish_proj(m, lo)

            # ---- cell 0 for step s ----
            if s < T:
                a0_sb = sb.tile([B, G], F32, tag="A0", bufs=3)
                dma_engines[s % 3].dma_start(a0_sb[:], a0_d[s])
                psr0, psz0, _ = gate_pair(h0T_fn, u0_s)
                h0T_new = sb.tile([128, KH, B], BF16, tag="hTl0", bufs=2)
                h0_s = cell_rest(
                    "l0", psr0, psz0, a0_sb[:], h0_s, u0_s, sc0,
                    lambda hf, src: nc.scalar.activation(
                        h0T_new[:, hf * 2 : hf * 2 + 2, :], src, AF.Copy
                    ),
                )
                h0T_fn = (lambda tl: lambda k: tl[:, k, :])(h0T_new)

                # ---- A1[s] = h0[s] @ W1f (+b1): filler for this slot ----
                A1_s = sb.tile([B, G], F32, tag="A1", bufs=2)
                for n in range(3):
                    ns = slice(n * 512, (n + 1) * 512)
                    psa = psA.tile([B, 512], F32, tag="psA")
                    for ki in range(KH):
                        nc.tensor.matmul(
                            psa[:], h0T_fn(ki), w1f_s[:, ki, ns],
                            start=(ki == 0), stop=(ki == KH - 1),
                        )
                    if n == 1:
                        nc.vector.tensor_scalar_add(A1_s[:, ns], psa[:], sc1["b1g"])
                    else:
                        nc.scalar.activation(
                            A1_s[:, ns], psa[:], AF.Identity,
                            bias=bias_ap(sc1["b1g"] if n == 0 else sc1["b1c"]),
                        )
                A1_prev = A1_s

        for m in tail_tiles:
            lo = sb.tile([128, VS], F32, tag="lout", bufs=2)
            emit_proj_bank(m, 0, lo, None)
            emit_proj_bank(m, 1, lo, None)
            finish_proj(m, lo)

        for p in (sb, psT, psG, psA, cpool):
            p.release()

    return nc, build


def kernel(**inputs):
    global LAST_RESULTS
    inp = {k: np.asarray(v) for k, v in inputs.items()}

    # ---- host prep ----
    xs = np.asarray(inp["embedding"], np.float32)[np.asarray(inp["input_data"])]
    # xs: [B, T, E]

    W0f, U0, sc0 = _fold_layer(
        inp["W0"], inp["U0"], inp["b0"], inp["alpha0"], inp["beta1_0"], inp["beta2_0"]
    )
    W1f, U1, sc1 = _fold_layer(
        inp["W1"], inp["U1"], inp["b1"], inp["alpha1"], inp["beta1_1"], inp["beta2_1"]
    )
    for sc in (sc0, sc1):
        assert abs(sc["b1g"] - sc["b1c"]) < 1e-12, "A bias fold needs b1g == b1c"

    # layer-0 input projection on host: A0[t] = x[t] @ W0f + b1  ([T, B, G] f32)
    A0 = np.einsum("bte,eg->tbg", xs, W0f.astype(np.float32)) + np.float32(
        sc0["b1g"]
    )
    A0 = np.ascontiguousarray(A0, dtype=np.float32)

    u0c = np.ascontiguousarray(U0.reshape(KH, 128, G))
    w1c = np.ascontiguousarray(W1f.reshape(KH, 128, G))
    u1c = np.ascontiguousarray(U1.reshape(KH, 128, G))

    wsm = np.asarray(inp["softmax_w"], np.float32)  # [H, V]
    sb = np.asarray(inp["softmax_b"], np.float32)  # [V]

    nc, build = _build_program()
    with tile.TileContext(nc) as tc:
        build(tc, {"l0": sc0, "l1": sc1})

    base_map = {
        "a0": A0,
        "u0": _bf16(u0c),
        "w1f": _bf16(w1c),
        "u1": _bf16(u1c),
    }
    in_maps = []
    for c in range(NCORES):
        vs = slice(c * VS, (c + 1) * VS)
        m = dict(base_map)
        m["wsm"] = _bf16(np.ascontiguousarray(wsm[:, vs]).reshape(KH, 128, VS))
        m["sbr"] = np.ascontiguousarray(
            np.tile(sb[vs][None, :], (128, 1)).astype(np.float32)
        )
        in_maps.append(m)

    from concourse.bass_utils import run_bass_kernel_spmd

    trace = bool(int(os.environ.get("KERNEL_TRACE", "0")))
    res = run_bass_kernel_spmd(
        nc, in_maps, core_ids=list(range(NCORES)), trace=trace
    )
    LAST_RESULTS = res

    # ---- assemble: concat vocab slices, reorder rows (t-major -> b-major) ----
    logits_tb = np.concatenate(
        [res.results[c]["out"] for c in range(NCORES)], axis=1
    )  # [T*B, V]
    logits = (
        logits_tb.reshape(T, B, V).transpose(1, 0, 2).reshape(B * T, V)
    )
    return np.ascontiguousarray(logits.astype(np.float32))


# revision 16
# speedup vs baseline: 1.2695x; 1.1034x over previous
"""Trainium2 Bass kernel for nn_CharRNN: 2-layer MI-GRU + large vocab projection.

Strategy (8 NeuronCores, SPMD, no collectives):
  - The sequential GRU recurrence (T=50 steps, B=100) is replicated on all
    8 cores: per-step matmul time is weight-column bound (independent of B),
    so batch-sharding would not speed it up, and replication avoids any
    cross-core synchronization.
  - The output projection logits = out @ softmax_w + b ([5000, 8000], 160 MB)
    is sharded over the vocab axis: core i computes columns [i*1000, (i+1)*1000)
    and writes its own 20 MB slice. The projection is interleaved into the
    recurrence loop (one 100-row stripe per timestep) so its matmuls fill the
    Tensor-engine stalls of the recurrent dependency chain and there is no
    serial tail.
  - Layer-0's input projection A0 = x@W0*alpha + beta1 depends only on the
    embedded inputs, so it is computed on the host and DMA-streamed per step,
    removing its matmuls + PSUM evacuations from the device entirely.

Layouts:
  - Gate/elementwise tensors: [B=100 partitions, features free], bf16 state;
    gate pre-activations kept f32 (the sigmoid/tanh argument is a small
    residual of values near 1 - rounding before the -1 bias is catastrophic).
  - Matmuls: out[B, N] = lhsT.T @ rhs with stationary lhsT = transposed
    activations [K=128 chunk, B] (bf16) and moving rhs = weight columns
    (bf16, 1 col/cycle; fp32 would be 2 cyc/col). Hidden-state transposes
    done on the PE via identity matmul; the r-path is split in 256-col
    halves so transposes/candidate-matmul start earlier (chain pipelining).
  - alpha/beta1/beta2/b are folded on the host:
      gate = sig((a*wx + b1) * (uh + b2/a) + (b - b1*b2/a))
"""

import os
import sys

sys.path.insert(0, "/opt/trn_rl_repo")

import ml_dtypes
import numpy as np

import concourse.bass as bass
import concourse.mybir as mybir
import concourse.tile as tile
from concourse.masks import make_identity

# ----------------------------------------------------------------------------
# Patch: the final SP Drain emitted by TileContext collects one semaphore wait
# per busy logical processor, but the walrus build in this container only
# lowers a limited number of sync-wait commands per CTRL instruction.  Split
# the waits across separate single-wait NoOps.
# ----------------------------------------------------------------------------
from concourse.vector_clock import ScopedClock
from bass_rust import SyncInfo

_MAXW = 1


def _patched_drain_and_barrier(self, tick_clock, wait_clock):
    nc = self.nc
    drain_inst = nc.sync.drain()
    wait_clock.add_sem_waits(
        drain_inst.ins, ScopedClock({None: tick_clock.global_clock})
    )
    si = drain_inst.ins.sync_info
    waits = list(si.on_wait) if si is not None else []
    if len(waits) > _MAXW:
        drain_inst.ins.sync_info = SyncInfo(
            on_wait=waits[:_MAXW], on_update=list(si.on_update)
        )
        for k in range(_MAXW, len(waits), _MAXW):
            nop = nc.sync.nop(nofuse=True)
            nop.ins.sync_info = SyncInfo(on_wait=waits[k : k + _MAXW], on_update=[])

    nc.all_engine_barrier()
    assert self.sems is not None
    popped = nc._tile_sem_poison_stack.pop()
    assert popped is self._sem_poison
    nc.clear_and_free_semaphores(list(self.sems.allocated().values()))
    nc.all_engine_barrier()


tile.TileContext._drain_and_barrier = _patched_drain_and_barrier

# ----------------------------------------------------------------------------
# Same walrus limitation applies to every engine instruction: split any
# instruction carrying more than _JLIM semaphore waits into preceding
# single-wait NoOps on the same engine (engines are in-order, so blocking on
# a prior NoOp is equivalent).  Done as a BIR-JSON post-pass on serialization.
# ----------------------------------------------------------------------------
import json as _json

_JLIM = 1
_orig_to_json_bytes = bass.Bass.to_json_bytes


def _split_waits_json(self) -> bytes:
    raw = _orig_to_json_bytes(self)
    d = _json.loads(raw)
    ctr = [0]

    def fix_block(blk):
        insts = blk.get("instructions")
        if insts:
            out = []
            for ins in insts:
                si = ins.get("sync_info")
                waits = (si or {}).get("on_wait") or []
                if len(waits) > _JLIM:
                    keep = waits[:_JLIM]
                    extra = waits[_JLIM:]
                    for k in range(0, len(extra), _JLIM):
                        ctr[0] += 1
                        out.append(
                            {
                                "debug": ins.get("debug", 0),
                                "engine": ins["engine"],
                                "ins": [],
                                "name": f"I-sw{ctr[0]}",
                                "opcode": "NoOp",
                                "outs": [],
                                "sync_info": {
                                    "on_wait": extra[k : k + _JLIM],
                                    "on_update": [],
                                },
                            }
                        )
                    si["on_wait"] = keep
                out.append(ins)
            blk["instructions"] = out
        for sub in blk.get("blocks", []) or []:
            fix_block(sub)

    for f in d.get("functions", []):
        for blk in f.get("blocks", []) or []:
            fix_block(blk)
    return _json.dumps(d).encode()


bass.Bass.to_json_bytes = _split_waits_json

# ----------------------------------------------------------------------------

B, T, H, E, V = 100, 50, 512, 128, 8000
G = 3 * H  # 1536
NCORES = 8
VS = V // NCORES  # 1000 vocab columns per core
KH = H // 128  # 4 K-chunks for H contraction
ROWS = B * T  # 5000 output rows
BF16 = mybir.dt.bfloat16
F32 = mybir.dt.float32
AF = mybir.ActivationFunctionType
ALU = mybir.AluOpType

# stash for test.py introspection
LAST_RESULTS = None


def _const_scalar(row, name):
    row = np.asarray(row, dtype=np.float64)
    lo, hi = row.min(), row.max()
    assert hi - lo < 1e-12, f"{name} is not a constant row; fast path invalid"
    return float(row[0])


def _bf16(a):
    return np.ascontiguousarray(np.asarray(a, dtype=np.float32)).astype(
        ml_dtypes.bfloat16
    )


def _fold_layer(W, U, b, alpha, beta1, beta2):
    """Host folding of the MI-GRU cell constants.

    gate_arg = alpha*wx*uh + beta1*uh + beta2*wx + b
             = (alpha*wx + beta1) * (uh + beta2/alpha) + (b - beta1*beta2/alpha)
    """
    W, U = np.asarray(W, np.float64), np.asarray(U, np.float64)
    alpha = np.asarray(alpha, np.float64)
    beta1 = np.asarray(beta1, np.float64)
    beta2 = np.asarray(beta2, np.float64)
    b = np.asarray(b, np.float64)
    Wf = W * alpha[None, :]
    r2 = beta2 / alpha
    d = b - beta1 * beta2 / alpha
    # per-range scalars (rows are constant in this problem)
    sc = {
        "b1g": _const_scalar(beta1[: 2 * H], "beta1_g"),
        "b1c": _const_scalar(beta1[2 * H :], "beta1_c"),
        "r2g": _const_scalar(r2[: 2 * H], "r2_g"),
        "r2c": _const_scalar(r2[2 * H :], "r2_c"),
        "dg": _const_scalar(d[: 2 * H], "d_g"),
        "dc": _const_scalar(d[2 * H :], "d_c"),
    }
    return Wf.astype(np.float32), np.asarray(U, np.float32), sc


def _build_program():
    nc = bass.Bass(
        "TRN2", target_bir_lowering=False, debug=False, num_devices=NCORES
    )

    # DRAM I/O (all recurrence weights bf16: fp32 moving operands stream at
    # 2 cycles/col on the PE, bf16 at 1)
    a0_d = nc.dram_tensor("a0", [T, B, G], F32, kind="ExternalInput").ap()
    u0_d = nc.dram_tensor("u0", [KH, 128, G], BF16, kind="ExternalInput").ap()
    w1f_d = nc.dram_tensor("w1f", [KH, 128, G], BF16, kind="ExternalInput").ap()
    u1_d = nc.dram_tensor("u1", [KH, 128, G], BF16, kind="ExternalInput").ap()
    wsm_d = nc.dram_tensor("wsm", [KH, 128, VS], BF16, kind="ExternalInput").ap()
    sbr_d = nc.dram_tensor("sbr", [128, VS], F32, kind="ExternalInput").ap()
    out_d = nc.dram_tensor("out", [ROWS, VS], F32, kind="ExternalOutput").ap()

    def build(tc, sc):
        nc = tc.nc
        cpool = tc.alloc_tile_pool(name="const", bufs=1)
        ld_engs = [nc.sync, nc.gpsimd, nc.scalar]
        # recurrence weights first: the first steps need them
        u0_s = cpool.tile([128, KH, G], BF16, tag="u0")
        w1f_s = cpool.tile([128, KH, G], BF16, tag="w1f")
        u1_s = cpool.tile([128, KH, G], BF16, tag="u1")
        for k in range(KH):
            ld_engs[k % 3].dma_start(u0_s[:, k, :], u0_d[k])
        for k in range(KH):
            ld_engs[(k + 1) % 3].dma_start(w1f_s[:, k, :], w1f_d[k])
            ld_engs[(k + 2) % 3].dma_start(u1_s[:, k, :], u1_d[k])
        wsm_s = cpool.tile([128, KH, VS], BF16, tag="wsm")
        for k in range(KH):
            ld_engs[k % 3].dma_start(wsm_s[:, k, :], wsm_d[k])
        sbr_s = cpool.tile([128, VS], F32, tag="sbr")
        nc.sync.dma_start(sbr_s[:], sbr_d[:])

        ident = cpool.tile([128, 128], BF16, tag="ident")
        make_identity(nc, ident[:])

        # bias constant tiles for ACT activations (bias must be an AP)
        _bias_tiles = {}

        def bias_ap(val, parts=B):
            val = float(val)
            if val not in _bias_tiles:
                bt = cpool.tile([128, 1], F32, tag=f"bias_{len(_bias_tiles)}")
                nc.vector.memset(bt[:], val)
                _bias_tiles[val] = bt
            return _bias_tiles[val][:parts]

        # initial states (zeros, bf16)
        h0_s = cpool.tile([B, H], BF16, tag="h0_init")
        h1_s = cpool.tile([B, H], BF16, tag="h1_init")
        h0T = cpool.tile([128, KH, B], BF16, tag="h0T_init")
        h1T = cpool.tile([128, KH, B], BF16, tag="h1T_init")
        nc.vector.memset(h0_s[:], 0.0)
        nc.vector.memset(h1_s[:], 0.0)
        nc.vector.memset(h0T[:], 0.0)
        nc.vector.memset(h1T[:], 0.0)

        # pools: PSUM banks: psA 1 + psG 4 + psT 2 + psP 1 = 8
        psA = tc.alloc_tile_pool(name="psA", bufs=1, space="PSUM")
        psG = tc.alloc_tile_pool(name="psG", bufs=4, space="PSUM")
        psT = tc.alloc_tile_pool(name="psT", bufs=2, space="PSUM")
        psP = tc.alloc_tile_pool(name="psP", bufs=1, space="PSUM")
        sb = tc.alloc_tile_pool(name="sb", bufs=2)

        # persistent bank of transposed layer-1 states: enables 128-row
        # projection M-tiles spanning step boundaries
        h1T_all = cpool.tile([128, KH, ROWS], BF16, tag="h1T_all")

        # PE warm-up: dummy transposes during the initial weight DMAs keep
        # the HAM clock-gate at 8/8 so step 0 starts at full clock
        wt = psT.tile([128, 128], BF16, tag="psT")
        for _ in range(40):
            nc.tensor.transpose(wt[:], ident[:], ident[:])

        def gate_pair(hT_fn, U_s):
            """Emit the r/z gate matmul groups (ready at cell start)."""
            psr = psG.tile([B, 512], F32, tag="psG")
            for k in range(KH):
                nc.tensor.matmul(
                    psr[:], hT_fn(k), U_s[:, k, 0:512],
                    start=(k == 0), stop=(k == KH - 1),
                )
            psz = psG.tile([B, 512], F32, tag="psG")
            last = None
            for k in range(KH):
                last = nc.tensor.matmul(
                    psz[:], hT_fn(k), U_s[:, k, 512:1024],
                    start=(k == 0), stop=(k == KH - 1),
                )
            return psr, psz, last

        def cell_rest(lt, psr, psz, A_ap, h_prev, U_s, sc_l, hT_evac):
            """Everything after the r/z matmuls. hT_evac(hf, src_ap) stores
            the transposed new state's half hf. Returns new_h [B,H] bf16."""
            # --- r path, split in 256-col halves for chain pipelining ---
            rh = sb.tile([B, 512], BF16, tag=f"rh{lt}", bufs=3)
            pst_r = psT.tile([128, KH, B], BF16, tag="psT")
            rhT = sb.tile([128, KH, B], BF16, tag=f"rhT{lt}", bufs=3)
            for hf in range(2):
                hs = slice(hf * 256, (hf + 1) * 256)
                m_r = sb.tile([B, 256], F32, tag=f"mr{lt}{hf}", bufs=2)
                nc.vector.scalar_tensor_tensor(
                    m_r[:], psr[:, hs], sc_l["r2g"], A_ap[:, hs], ALU.add, ALU.mult
                )
                r = sb.tile([B, 256], BF16, tag=f"r{lt}{hf}", bufs=3)
                nc.scalar.activation(
                    r[:], m_r[:], AF.Sigmoid, bias=bias_ap(sc_l["dg"])
                )
                nc.vector.tensor_mul(rh[:, hs], r[:], h_prev[:, hs])
                for jj in range(2):
                    j = hf * 2 + jj
                    nc.tensor.transpose(
                        pst_r[:, j, :], rh[:, j * 128 : (j + 1) * 128],
                        ident[:B, :B],
                    )
                nc.scalar.activation(
                    rhT[:, hf * 2 : hf * 2 + 2, :],
                    pst_r[:, hf * 2 : hf * 2 + 2, :],
                    AF.Copy,
                )

            # --- candidate: psc accumulates over rhT chunks ---
            psc = psG.tile([B, 512], F32, tag="psG")
            for k in range(KH):
                nc.tensor.matmul(
                    psc[:], rhT[:, k, :], U_s[:, k, 1024:1536],
                    start=(k == 0), stop=(k == KH - 1),
                )

            # --- z gate elementwise (early: q needs z at the tail) ---
            m_z = sb.tile([B, 512], F32, tag=f"mz{lt}", bufs=2)
            nc.vector.scalar_tensor_tensor(
                m_z[:], psz[:], sc_l["r2g"], A_ap[:, 512:1024], ALU.add, ALU.mult
            )
            z = sb.tile([B, 512], BF16, tag=f"z{lt}", bufs=3)
            nc.scalar.activation(z[:], m_z[:], AF.Sigmoid, bias=bias_ap(sc_l["dg"]))
            zh = sb.tile([B, 512], BF16, tag=f"zh{lt}", bufs=3)
            nc.vector.tensor_mul(zh[:], z[:], h_prev[:])

            # --- c path + combine, split in halves: new_h = zh - (z-1)*c ---
            nh = sb.tile([B, H], BF16, tag=f"h{lt}", bufs=3)
            pst_h = psT.tile([128, KH, B], BF16, tag="psT")
            for hf in range(2):
                hs = slice(hf * 256, (hf + 1) * 256)
                m_c = sb.tile([B, 256], F32, tag=f"mc{lt}{hf}", bufs=3)
                nc.vector.scalar_tensor_tensor(
                    m_c[:], psc[:, hs], sc_l["r2c"],
                    A_ap[:, 1024 + hf * 256 : 1024 + (hf + 1) * 256],
                    ALU.add, ALU.mult,
                )
                cc = sb.tile([B, 256], BF16, tag=f"c{lt}{hf}", bufs=3)
                nc.scalar.activation(
                    cc[:], m_c[:], AF.Tanh, bias=bias_ap(sc_l["dc"])
                )
                q = sb.tile([B, 256], BF16, tag=f"q{lt}{hf}", bufs=3)
                nc.vector.scalar_tensor_tensor(
                    q[:], z[:, hs], 1.0, cc[:], ALU.subtract, ALU.mult
                )
                nc.vector.tensor_tensor(nh[:, hs], zh[:, hs], q[:], ALU.subtract)
                for jj in range(2):
                    j = hf * 2 + jj
                    nc.tensor.transpose(
                        pst_h[:, j, :], nh[:, j * 128 : (j + 1) * 128],
                        ident[:B, :B],
                    )
                hT_evac(hf, pst_h[:, hf * 2 : hf * 2 + 2, :])
            return nh

        dma_engines = [nc.sync, nc.gpsimd, nc.scalar]

        def emit_proj_bank(m, nb, lo, dep_ins):
            mrows = min(128, ROWS - m * 128)
            ns = slice(nb * 500, (nb + 1) * 500)
            psp = psP.tile([128, 500], F32, tag="psP")
            for k in range(KH):
                mm = nc.tensor.matmul(
                    psp[:mrows, :],
                    h1T_all[:, k, m * 128 : m * 128 + mrows],
                    wsm_s[:, k, ns],
                    start=(k == 0), stop=(k == KH - 1),
                )
                if k == 0 and dep_ins is not None:
                    tile.add_dep_helper(
                        mm.ins, dep_ins, reason="delay proj into chain window"
                    )
            nc.vector.tensor_add(
                lo[:mrows, ns], psp[:mrows, :], sbr_s[:mrows, ns]
            )

        def finish_proj(m, lo):
            mrows = min(128, ROWS - m * 128)
            dma_engines[m % 3].dma_start(
                out_d[m * 128 : m * 128 + mrows, :], lo[:mrows, :]
            )

        # projection M-tile emission schedule (cell1 lags one slot):
        # h1T[t] is produced in slot t+1, so tile m is emittable in slot
        # (last_row//B) + 2
        proj_emit = {}
        tail_tiles = []
        for m in range((ROWS + 127) // 128):
            last_row = min((m + 1) * 128, ROWS) - 1
            t_emit = last_row // B + 2
            if t_emit <= T:
                proj_emit.setdefault(t_emit, []).append(m)
            else:
                tail_tiles.append(m)

        sc0, sc1 = sc["l0"], sc["l1"]
        h0T_fn = lambda k: h0T[:, k, :]
        h1T_fn = lambda k: h1T[:, k, :]
        A1_prev = None
        # Slot s runs cell0[s] and, fully independently, cell1[s-1]
        # (whose A1 input was computed last slot as gap filler).
        for s in range(T + 1):
            t1 = s - 1  # cell1's timestep this slot

            # ---- cell 1 for step t1: all inputs ready at slot start ----
            if t1 >= 0:
                psr1, psz1, _ = gate_pair(h1T_fn, u1_s)
                h1_s = cell_rest(
                    "l1", psr1, psz1, A1_prev[:], h1_s, u1_s, sc1,
                    (lambda t_: lambda hf, src: nc.scalar.activation(
                        h1T_all[:, hf * 2 : hf * 2 + 2, t_ * B : (t_ + 1) * B],
                        src, AF.Copy,
                    ))(t1),
                )
                h1T_fn = (lambda t_: lambda k: h1T_all[:, k, t_ * B : (t_ + 1) * B])(t1)

            # ---- projection tiles due this slot (pure filler) ----
            for m in proj_emit.get(s, []):
                lo = sb.tile([128, VS], F32, tag="lout", bufs=2)
                emit_proj_bank(m, 0, lo, None)
                emit_proj_bank(m, 1, lo, None)
                finish_proj(m, lo)

            # ---- cell 0 for step s ----
            if s < T:
                a0_sb = sb.tile([B, G], F32, tag="A0", bufs=3)
                dma_engines[s % 3].dma_start(a0_sb[:], a0_d[s])
                psr0, psz0, _ = gate_pair(h0T_fn, u0_s)
                h0T_new = sb.tile([128, KH, B], BF16, tag="hTl0", bufs=3)
                h0_s = cell_rest(
                    "l0", psr0, psz0, a0_sb[:], h0_s, u0_s, sc0,
                    lambda hf, src: nc.scalar.activation(
                        h0T_new[:, hf * 2 : hf * 2 + 2, :], src, AF.Copy
                    ),
                )
                h0T_fn = (lambda tl: lambda k: tl[:, k, :])(h0T_new)

                # ---- A1[s] = h0[s] @ W1f (+b1): filler for this slot ----
                A1_s = sb.tile([B, G], F32, tag="A1", bufs=2)
                for n in range(3):
                    ns = slice(n * 512, (n + 1) * 512)
                    psa = psA.tile([B, 512], F32, tag="psA")
                    for ki in range(KH):
                        nc.tensor.matmul(
                            psa[:], h0T_fn(ki), w1f_s[:, ki, ns],
                            start=(ki == 0), stop=(ki == KH - 1),
                        )
                    if n == 1:
                        nc.vector.tensor_scalar_add(A1_s[:, ns], psa[:], sc1["b1g"])
                    else:
                        nc.scalar.activation(
                            A1_s[:, ns], psa[:], AF.Identity,
                            bias=bias_ap(sc1["b1g"] if n == 0 else sc1["b1c"]),
                        )
                A1_prev = A1_s

        for m in tail_tiles:
            lo = sb.tile([128, VS], F32, tag="lout", bufs=2)
            emit_proj_bank(m, 0, lo, None)
            emit_proj_bank(m, 1, lo, None)
            finish_proj(m, lo)

        for p in (sb, psP, psT, psG, psA, cpool):
            p.release()

    return nc, build


def kernel(**inputs):
    global LAST_RESULTS
    inp = {k: np.asarray(v) for k, v in inputs.items()}

    # ---- host prep ----
    xs = np.asarray(inp["embedding"], np.float32)[np.asarray(inp["input_data"])]
    # xs: [B, T, E]

    W0f, U0, sc0 = _fold_layer(
        inp["W0"], inp["U0"], inp["b0"], inp["alpha0"], inp["beta1_0"], inp["beta2_0"]
    )
    W1f, U1, sc1 = _fold_layer(
        inp["W1"], inp["U1"], inp["b1"], inp["alpha1"], inp["beta1_1"], inp["beta2_1"]
    )
    for sc in (sc0, sc1):
        assert abs(sc["b1g"] - sc["b1c"]) < 1e-12, "A bias fold needs b1g == b1c"

    # layer-0 input projection on host: A0[t] = x[t] @ W0f + b1  ([T, B, G] f32)
    A0 = np.einsum("bte,eg->tbg", xs, W0f.astype(np.float32)) + np.float32(
        sc0["b1g"]
    )
    A0 = np.ascontiguousarray(A0, dtype=np.float32)

    u0c = np.ascontiguousarray(U0.reshape(KH, 128, G))
    w1c = np.ascontiguousarray(W1f.reshape(KH, 128, G))
    u1c = np.ascontiguousarray(U1.reshape(KH, 128, G))

    wsm = np.asarray(inp["softmax_w"], np.float32)  # [H, V]
    sb = np.asarray(inp["softmax_b"], np.float32)  # [V]

    nc, build = _build_program()
    with tile.TileContext(nc) as tc:
        build(tc, {"l0": sc0, "l1": sc1})

    base_map = {
        "a0": A0,
        "u0": _bf16(u0c),
        "w1f": _bf16(w1c),
        "u1": _bf16(u1c),
    }
    in_maps = []
    for c in range(NCORES):
        vs = slice(c * VS, (c + 1) * VS)
        m = dict(base_map)
        m["wsm"] = _bf16(np.ascontiguousarray(wsm[:, vs]).reshape(KH, 128, VS))
        m["sbr"] = np.ascontiguousarray(
            np.tile(sb[vs][None, :], (128, 1)).astype(np.float32)
        )
        in_maps.append(m)

    from concourse.bass_utils import run_bass_kernel_spmd

    trace = bool(int(os.environ.get("KERNEL_TRACE", "0")))
    res = run_bass_kernel_spmd(
        nc, in_maps, core_ids=list(range(NCORES)), trace=trace
    )
    LAST_RESULTS = res

    # ---- assemble: concat vocab slices, reorder rows (t-major -> b-major) ----
    logits_tb = np.concatenate(
        [res.results[c]["out"] for c in range(NCORES)], axis=1
    )  # [T*B, V]
    logits = (
        logits_tb.reshape(T, B, V).transpose(1, 0, 2).reshape(B * T, V)
    )
    return np.ascontiguousarray(logits.astype(np.float32))


# revision 17
# speedup vs baseline: 1.2768x; 1.0058x over previous
"""Trainium2 Bass kernel for nn_CharRNN: 2-layer MI-GRU + large vocab projection.

Strategy (8 NeuronCores, SPMD, no collectives):
  - The sequential GRU recurrence (T=50 steps, B=100) is replicated on all
    8 cores: per-step matmul time is weight-column bound (independent of B),
    so batch-sharding would not speed it up, and replication avoids any
    cross-core synchronization.
  - The output projection logits = out @ softmax_w + b ([5000, 8000], 160 MB)
    is sharded over the vocab axis: core i computes columns [i*1000, (i+1)*1000)
    and writes its own 20 MB slice. The projection is computed as 128-row
    M-tiles from a persistent bank of transposed layer-1 states, interleaved
    into the recurrence loop so its matmuls fill Tensor-engine stalls of the
    recurrent dependency chain and there is no serial tail.
  - Layer-1 lags layer-0 by one timestep slot: cell1[t-1] runs concurrently
    with cell0[t], with A1[t-1] = h0[t-1] @ W1 computed the slot before as
    gap filler, so the two cells' dependency chains are fully independent
    within a slot.
  - Layer-0's input projection A0 = x@W0*alpha + beta1 depends only on the
    embedded inputs, so it is computed on the host and DMA-streamed per step,
    removing its matmuls + PSUM evacuations from the device entirely.

Layouts:
  - Gate/elementwise tensors: [B=100 partitions, features free], bf16 state;
    gate pre-activations kept f32 (the sigmoid/tanh argument is a small
    residual of values near 1 - rounding before the -1 bias is catastrophic).
  - Matmuls: out[B, N] = lhsT.T @ rhs with stationary lhsT = transposed
    activations [K=128 chunk, B] (bf16) and moving rhs = weight columns
    (bf16, 1 col/cycle; fp32 would be 2 cyc/col). Hidden-state transposes
    done on the PE via identity matmul; the r-path is split in 256-col
    halves so transposes/candidate-matmul start earlier (chain pipelining).
  - alpha/beta1/beta2/b are folded on the host:
      gate = sig((a*wx + b1) * (uh + b2/a) + (b - b1*b2/a))
"""

import os
import sys

sys.path.insert(0, "/opt/trn_rl_repo")

import ml_dtypes
import numpy as np

import concourse.bass as bass
import concourse.mybir as mybir
import concourse.tile as tile
from concourse.masks import make_identity

# ----------------------------------------------------------------------------
# Patch: the final SP Drain emitted by TileContext collects one semaphore wait
# per busy logical processor, but the walrus build in this container only
# lowers a limited number of sync-wait commands per CTRL instruction.  Split
# the waits across separate single-wait NoOps.
# ----------------------------------------------------------------------------
from concourse.vector_clock import ScopedClock
from bass_rust import SyncInfo

_MAXW = 1


def _patched_drain_and_barrier(self, tick_clock, wait_clock):
    nc = self.nc
    drain_inst = nc.sync.drain()
    wait_clock.add_sem_waits(
        drain_inst.ins, ScopedClock({None: tick_clock.global_clock})
    )
    si = drain_inst.ins.sync_info
    waits = list(si.on_wait) if si is not None else []
    if len(waits) > _MAXW:
        drain_inst.ins.sync_info = SyncInfo(
            on_wait=waits[:_MAXW], on_update=list(si.on_update)
        )
        for k in range(_MAXW, len(waits), _MAXW):
            nop = nc.sync.nop(nofuse=True)
            nop.ins.sync_info = SyncInfo(on_wait=waits[k : k + _MAXW], on_update=[])

    nc.all_engine_barrier()
    assert self.sems is not None
    popped = nc._tile_sem_poison_stack.pop()
    assert popped is self._sem_poison
    nc.clear_and_free_semaphores(list(self.sems.allocated().values()))
    nc.all_engine_barrier()


tile.TileContext._drain_and_barrier = _patched_drain_and_barrier

# ----------------------------------------------------------------------------
# Same walrus limitation applies to every engine instruction: split any
# instruction carrying more than _JLIM semaphore waits into preceding
# single-wait NoOps on the same engine (engines are in-order, so blocking on
# a prior NoOp is equivalent).  Done as a BIR-JSON post-pass on serialization.
# ----------------------------------------------------------------------------
import json as _json

_JLIM = 1
_orig_to_json_bytes = bass.Bass.to_json_bytes


def _split_waits_json(self) -> bytes:
    raw = _orig_to_json_bytes(self)
    d = _json.loads(raw)
    ctr = [0]

    def fix_block(blk):
        insts = blk.get("instructions")
        if insts:
            out = []
            for ins in insts:
                si = ins.get("sync_info")
                waits = (si or {}).get("on_wait") or []
                if len(waits) > _JLIM:
                    keep = waits[:_JLIM]
                    extra = waits[_JLIM:]
                    for k in range(0, len(extra), _JLIM):
                        ctr[0] += 1
                        out.append(
                            {
                                "debug": ins.get("debug", 0),
                                "engine": ins["engine"],
                                "ins": [],
                                "name": f"I-sw{ctr[0]}",
                                "opcode": "NoOp",
                                "outs": [],
                                "sync_info": {
                                    "on_wait": extra[k : k + _JLIM],
                                    "on_update": [],
                                },
                            }
                        )
                    si["on_wait"] = keep
                out.append(ins)
            blk["instructions"] = out
        for sub in blk.get("blocks", []) or []:
            fix_block(sub)

    for f in d.get("functions", []):
        for blk in f.get("blocks", []) or []:
            fix_block(blk)
    return _json.dumps(d).encode()


bass.Bass.to_json_bytes = _split_waits_json

# ----------------------------------------------------------------------------

B, T, H, E, V = 100, 50, 512, 128, 8000
G = 3 * H  # 1536
NCORES = 8
VS = V // NCORES  # 1000 vocab columns per core
KH = H // 128  # 4 K-chunks for H contraction
ROWS = B * T  # 5000 output rows
BF16 = mybir.dt.bfloat16
F32 = mybir.dt.float32
AF = mybir.ActivationFunctionType
ALU = mybir.AluOpType

# stash for test.py introspection
LAST_RESULTS = None


def _const_scalar(row, name):
    row = np.asarray(row, dtype=np.float64)
    lo, hi = row.min(), row.max()
    assert hi - lo < 1e-12, f"{name} is not a constant row; fast path invalid"
    return float(row[0])


def _bf16(a):
    return np.ascontiguousarray(np.asarray(a, dtype=np.float32)).astype(
        ml_dtypes.bfloat16
    )


def _fold_layer(W, U, b, alpha, beta1, beta2):
    """Host folding of the MI-GRU cell constants.

    gate_arg = alpha*wx*uh + beta1*uh + beta2*wx + b
             = (alpha*wx + beta1) * (uh + beta2/alpha) + (b - beta1*beta2/alpha)
    """
    W, U = np.asarray(W, np.float64), np.asarray(U, np.float64)
    alpha = np.asarray(alpha, np.float64)
    beta1 = np.asarray(beta1, np.float64)
    beta2 = np.asarray(beta2, np.float64)
    b = np.asarray(b, np.float64)
    Wf = W * alpha[None, :]
    r2 = beta2 / alpha
    d = b - beta1 * beta2 / alpha
    # per-range scalars (rows are constant in this problem)
    sc = {
        "b1g": _const_scalar(beta1[: 2 * H], "beta1_g"),
        "b1c": _const_scalar(beta1[2 * H :], "beta1_c"),
        "r2g": _const_scalar(r2[: 2 * H], "r2_g"),
        "r2c": _const_scalar(r2[2 * H :], "r2_c"),
        "dg": _const_scalar(d[: 2 * H], "d_g"),
        "dc": _const_scalar(d[2 * H :], "d_c"),
    }
    return Wf.astype(np.float32), np.asarray(U, np.float32), sc


def _build_program():
    nc = bass.Bass(
        "TRN2", target_bir_lowering=False, debug=False, num_devices=NCORES
    )

    # DRAM I/O (all recurrence weights bf16: fp32 moving operands stream at
    # 2 cycles/col on the PE, bf16 at 1)
    a0_d = nc.dram_tensor("a0", [T, B, G], F32, kind="ExternalInput").ap()
    u0_d = nc.dram_tensor("u0", [KH, 128, G], BF16, kind="ExternalInput").ap()
    w1f_d = nc.dram_tensor("w1f", [KH, 128, G], BF16, kind="ExternalInput").ap()
    u1_d = nc.dram_tensor("u1", [KH, 128, G], BF16, kind="ExternalInput").ap()
    wsm_d = nc.dram_tensor("wsm", [KH, 128, VS], BF16, kind="ExternalInput").ap()
    sbr_d = nc.dram_tensor("sbr", [128, VS], F32, kind="ExternalInput").ap()
    out_d = nc.dram_tensor("out", [ROWS, VS], F32, kind="ExternalOutput").ap()

    def build(tc, sc):
        nc = tc.nc
        cpool = tc.alloc_tile_pool(name="const", bufs=1)
        ld_engs = [nc.sync, nc.gpsimd, nc.scalar]
        # recurrence weights first: the first steps need them
        u0_s = cpool.tile([128, KH, G], BF16, tag="u0")
        w1f_s = cpool.tile([128, KH, G], BF16, tag="w1f")
        u1_s = cpool.tile([128, KH, G], BF16, tag="u1")
        for k in range(KH):
            ld_engs[k % 3].dma_start(u0_s[:, k, :], u0_d[k])
        for k in range(KH):
            ld_engs[(k + 1) % 3].dma_start(w1f_s[:, k, :], w1f_d[k])
            ld_engs[(k + 2) % 3].dma_start(u1_s[:, k, :], u1_d[k])
        wsm_s = cpool.tile([128, KH, VS], BF16, tag="wsm")
        for k in range(KH):
            ld_engs[k % 3].dma_start(wsm_s[:, k, :], wsm_d[k])
        sbr_s = cpool.tile([128, VS], F32, tag="sbr")
        nc.sync.dma_start(sbr_s[:], sbr_d[:])

        ident = cpool.tile([128, 128], BF16, tag="ident")
        make_identity(nc, ident[:])

        # bias constant tiles for ACT activations (bias must be an AP)
        _bias_tiles = {}

        def bias_ap(val, parts=B):
            val = float(val)
            if val not in _bias_tiles:
                bt = cpool.tile([128, 1], F32, tag=f"bias_{len(_bias_tiles)}")
                nc.vector.memset(bt[:], val)
                _bias_tiles[val] = bt
            return _bias_tiles[val][:parts]

        # initial states (zeros, bf16)
        h0_s = cpool.tile([B, H], BF16, tag="h0_init")
        h1_s = cpool.tile([B, H], BF16, tag="h1_init")
        h0T = cpool.tile([128, KH, B], BF16, tag="h0T_init")
        h1T = cpool.tile([128, KH, B], BF16, tag="h1T_init")
        nc.vector.memset(h0_s[:], 0.0)
        nc.vector.memset(h1_s[:], 0.0)
        nc.vector.memset(h0T[:], 0.0)
        nc.vector.memset(h1T[:], 0.0)

        # pools: PSUM banks: psA 1 + psG 4 + psT 2 + psP 1 = 8
        psA = tc.alloc_tile_pool(name="psA", bufs=1, space="PSUM")
        psG = tc.alloc_tile_pool(name="psG", bufs=4, space="PSUM")
        psT = tc.alloc_tile_pool(name="psT", bufs=2, space="PSUM")
        psP = tc.alloc_tile_pool(name="psP", bufs=1, space="PSUM")
        sb = tc.alloc_tile_pool(name="sb", bufs=2)

        # persistent bank of transposed layer-1 states: enables 128-row
        # projection M-tiles spanning step boundaries
        h1T_all = cpool.tile([128, KH, ROWS], BF16, tag="h1T_all")

        # PE warm-up: dummy transposes during the initial weight DMAs keep
        # the HAM clock-gate at 8/8 so step 0 starts at full clock
        wt = psT.tile([128, 128], BF16, tag="psT")
        for _ in range(40):
            nc.tensor.transpose(wt[:], ident[:], ident[:])

        def gate_pair(hT_fn, U_s):
            """Emit the r/z gate matmul groups (ready at cell start)."""
            psr = psG.tile([B, 512], F32, tag="psG")
            for k in range(KH):
                nc.tensor.matmul(
                    psr[:], hT_fn(k), U_s[:, k, 0:512],
                    start=(k == 0), stop=(k == KH - 1),
                )
            psz = psG.tile([B, 512], F32, tag="psG")
            last = None
            for k in range(KH):
                last = nc.tensor.matmul(
                    psz[:], hT_fn(k), U_s[:, k, 512:1024],
                    start=(k == 0), stop=(k == KH - 1),
                )
            return psr, psz, last

        def cell_rest(lt, psr, psz, A_ap, h_prev, U_s, sc_l, hT_evac):
            """Everything after the r/z matmuls. hT_evac(hf, src_ap) stores
            the transposed new state's half hf. Returns new_h [B,H] bf16."""
            # --- r path, split in 256-col halves for chain pipelining ---
            rh = sb.tile([B, 512], BF16, tag=f"rh{lt}", bufs=3)
            pst_r = psT.tile([128, KH, B], BF16, tag="psT")
            rhT = sb.tile([128, KH, B], BF16, tag=f"rhT{lt}", bufs=3)
            for hf in range(2):
                hs = slice(hf * 256, (hf + 1) * 256)
                m_r = sb.tile([B, 256], F32, tag=f"mr{lt}{hf}", bufs=2)
                nc.vector.scalar_tensor_tensor(
                    m_r[:], psr[:, hs], sc_l["r2g"], A_ap[:, hs], ALU.add, ALU.mult
                )
                r = sb.tile([B, 256], BF16, tag=f"r{lt}{hf}", bufs=3)
                nc.scalar.activation(
                    r[:], m_r[:], AF.Sigmoid, bias=bias_ap(sc_l["dg"])
                )
                nc.vector.tensor_mul(rh[:, hs], r[:], h_prev[:, hs])
                for jj in range(2):
                    j = hf * 2 + jj
                    nc.tensor.transpose(
                        pst_r[:, j, :], rh[:, j * 128 : (j + 1) * 128],
                        ident[:B, :B],
                    )
                nc.scalar.activation(
                    rhT[:, hf * 2 : hf * 2 + 2, :],
                    pst_r[:, hf * 2 : hf * 2 + 2, :],
                    AF.Copy,
                )

            # --- candidate: psc accumulates over rhT chunks ---
            psc = psG.tile([B, 512], F32, tag="psG")
            for k in range(KH):
                nc.tensor.matmul(
                    psc[:], rhT[:, k, :], U_s[:, k, 1024:1536],
                    start=(k == 0), stop=(k == KH - 1),
                )

            # --- z gate elementwise (early: q needs z at the tail) ---
            m_z = sb.tile([B, 512], F32, tag=f"mz{lt}", bufs=2)
            nc.vector.scalar_tensor_tensor(
                m_z[:], psz[:], sc_l["r2g"], A_ap[:, 512:1024], ALU.add, ALU.mult
            )
            z = sb.tile([B, 512], BF16, tag=f"z{lt}", bufs=3)
            nc.scalar.activation(z[:], m_z[:], AF.Sigmoid, bias=bias_ap(sc_l["dg"]))
            zh = sb.tile([B, 512], BF16, tag=f"zh{lt}", bufs=3)
            nc.vector.tensor_mul(zh[:], z[:], h_prev[:])

            # --- c path + combine, split in halves: new_h = zh - (z-1)*c ---
            nh = sb.tile([B, H], BF16, tag=f"h{lt}", bufs=3)
            pst_h = psT.tile([128, KH, B], BF16, tag="psT")
            for hf in range(2):
                hs = slice(hf * 256, (hf + 1) * 256)
                m_c = sb.tile([B, 256], F32, tag=f"mc{lt}{hf}", bufs=3)
                nc.vector.scalar_tensor_tensor(
                    m_c[:], psc[:, hs], sc_l["r2c"],
                    A_ap[:, 1024 + hf * 256 : 1024 + (hf + 1) * 256],
                    ALU.add, ALU.mult,
                )
                cc = sb.tile([B, 256], BF16, tag=f"c{lt}{hf}", bufs=3)
                nc.scalar.activation(
                    cc[:], m_c[:], AF.Tanh, bias=bias_ap(sc_l["dc"])
                )
                q = sb.tile([B, 256], BF16, tag=f"q{lt}{hf}", bufs=3)
                nc.vector.scalar_tensor_tensor(
                    q[:], z[:, hs], 1.0, cc[:], ALU.subtract, ALU.mult
                )
                nc.vector.tensor_tensor(nh[:, hs], zh[:, hs], q[:], ALU.subtract)
                for jj in range(2):
                    j = hf * 2 + jj
                    nc.tensor.transpose(
                        pst_h[:, j, :], nh[:, j * 128 : (j + 1) * 128],
                        ident[:B, :B],
                    )
                hT_evac(hf, pst_h[:, hf * 2 : hf * 2 + 2, :])
            return nh

        dma_engines = [nc.sync, nc.gpsimd, nc.scalar]

        def emit_proj_bank(m, nb, lo, dep_ins):
            mrows = min(128, ROWS - m * 128)
            ns = slice(nb * 500, (nb + 1) * 500)
            psp = psP.tile([128, 500], F32, tag="psP")
            for k in range(KH):
                mm = nc.tensor.matmul(
                    psp[:mrows, :],
                    h1T_all[:, k, m * 128 : m * 128 + mrows],
                    wsm_s[:, k, ns],
                    start=(k == 0), stop=(k == KH - 1),
                )
                if k == 0 and dep_ins is not None:
                    tile.add_dep_helper(
                        mm.ins, dep_ins, reason="delay proj into chain window"
                    )
            nc.vector.tensor_add(
                lo[:mrows, ns], psp[:mrows, :], sbr_s[:mrows, ns]
            )

        def finish_proj(m, lo):
            mrows = min(128, ROWS - m * 128)
            dma_engines[m % 3].dma_start(
                out_d[m * 128 : m * 128 + mrows, :], lo[:mrows, :]
            )

        # projection M-tile emission schedule (cell1 lags one slot):
        # h1T[t] is produced in slot t+1, so tile m is emittable in slot
        # (last_row//B) + 2
        proj_emit = {}
        tail_tiles = []
        for m in range((ROWS + 127) // 128):
            last_row = min((m + 1) * 128, ROWS) - 1
            t_emit = last_row // B + 2
            if t_emit <= T:
                proj_emit.setdefault(t_emit, []).append(m)
            else:
                tail_tiles.append(m)

        sc0, sc1 = sc["l0"], sc["l1"]
        h0T_fn = lambda k: h0T[:, k, :]
        h1T_fn = lambda k: h1T[:, k, :]
        A1_prev = None
        # Slot s runs cell0[s] and, fully independently, cell1[s-1]
        # (whose A1 input was computed last slot as gap filler).
        for s in range(T + 1):
            t1 = s - 1  # cell1's timestep this slot

            # ---- cell 1 for step t1: all inputs ready at slot start ----
            if t1 >= 0:
                psr1, psz1, _ = gate_pair(h1T_fn, u1_s)
                h1_s = cell_rest(
                    "l1", psr1, psz1, A1_prev[:], h1_s, u1_s, sc1,
                    (lambda t_: lambda hf, src: nc.scalar.activation(
                        h1T_all[:, hf * 2 : hf * 2 + 2, t_ * B : (t_ + 1) * B],
                        src, AF.Copy,
                    ))(t1),
                )
                h1T_fn = (lambda t_: lambda k: h1T_all[:, k, t_ * B : (t_ + 1) * B])(t1)

            # ---- projection tiles due this slot (pure filler) ----
            for m in proj_emit.get(s, []):
                lo = sb.tile([128, VS], F32, tag="lout", bufs=2)
                emit_proj_bank(m, 0, lo, None)
                emit_proj_bank(m, 1, lo, None)
                finish_proj(m, lo)

            # ---- cell 0 for step s ----
            if s < T:
                a0_sb = sb.tile([B, G], F32, tag="A0", bufs=3)
                dma_engines[s % 3].dma_start(a0_sb[:], a0_d[s])
                psr0, psz0, _ = gate_pair(h0T_fn, u0_s)
                h0T_new = sb.tile([128, KH, B], BF16, tag="hTl0", bufs=3)
                h0_s = cell_rest(
                    "l0", psr0, psz0, a0_sb[:], h0_s, u0_s, sc0,
                    lambda hf, src: nc.scalar.activation(
                        h0T_new[:, hf * 2 : hf * 2 + 2, :], src, AF.Copy
                    ),
                )
                h0T_fn = (lambda tl: lambda k: tl[:, k, :])(h0T_new)

                # ---- A1[s] = h0[s] @ W1f (+b1): filler for this slot ----
                A1_s = sb.tile([B, G], F32, tag="A1", bufs=2)
                for n in range(3):
                    ns = slice(n * 512, (n + 1) * 512)
                    psa = psA.tile([B, 512], F32, tag="psA")
                    for ki in range(KH):
                        nc.tensor.matmul(
                            psa[:], h0T_fn(ki), w1f_s[:, ki, ns],
                            start=(ki == 0), stop=(ki == KH - 1),
                        )
                    if n == 1:
                        nc.vector.tensor_scalar_add(A1_s[:, ns], psa[:], sc1["b1g"])
                    else:
                        nc.scalar.activation(
                            A1_s[:, ns], psa[:], AF.Identity,
                            bias=bias_ap(sc1["b1g"] if n == 0 else sc1["b1c"]),
                        )
                A1_prev = A1_s

        for m in tail_tiles:
            lo = sb.tile([128, VS], F32, tag="lout", bufs=2)
            emit_proj_bank(m, 0, lo, None)
            emit_proj_bank(m, 1, lo, None)
            finish_proj(m, lo)

        for p in (sb, psP, psT, psG, psA, cpool):
            p.release()

    return nc, build


def kernel(**inputs):
    global LAST_RESULTS
    inp = {k: np.asarray(v) for k, v in inputs.items()}

    # ---- host prep ----
    xs = np.asarray(inp["embedding"], np.float32)[np.asarray(inp["input_data"])]
    # xs: [B, T, E]

    W0f, U0, sc0 = _fold_layer(
        inp["W0"], inp["U0"], inp["b0"], inp["alpha0"], inp["beta1_0"], inp["beta2_0"]
    )
    W1f, U1, sc1 = _fold_layer(
        inp["W1"], inp["U1"], inp["b1"], inp["alpha1"], inp["beta1_1"], inp["beta2_1"]
    )
    for sc in (sc0, sc1):
        assert abs(sc["b1g"] - sc["b1c"]) < 1e-12, "A bias fold needs b1g == b1c"

    # layer-0 input projection on host: A0[t] = x[t] @ W0f + b1  ([T, B, G] f32)
    A0 = np.einsum("bte,eg->tbg", xs, W0f.astype(np.float32)) + np.float32(
        sc0["b1g"]
    )
    A0 = np.ascontiguousarray(A0, dtype=np.float32)

    u0c = np.ascontiguousarray(U0.reshape(KH, 128, G))
    w1c = np.ascontiguousarray(W1f.reshape(KH, 128, G))
    u1c = np.ascontiguousarray(U1.reshape(KH, 128, G))

    wsm = np.asarray(inp["softmax_w"], np.float32)  # [H, V]
    sb = np.asarray(inp["softmax_b"], np.float32)  # [V]

    nc, build = _build_program()
    with tile.TileContext(nc) as tc:
        build(tc, {"l0": sc0, "l1": sc1})

    base_map = {
        "a0": A0,
        "u0": _bf16(u0c),
        "w1f": _bf16(w1c),
        "u1": _bf16(u1c),
    }
    in_maps = []
    for c in range(NCORES):
        vs = slice(c * VS, (c + 1) * VS)
        m = dict(base_map)
        m["wsm"] = _bf16(np.ascontiguousarray(wsm[:, vs]).reshape(KH, 128, VS))
        m["sbr"] = np.ascontiguousarray(
            np.tile(sb[vs][None, :], (128, 1)).astype(np.float32)
        )
        in_maps.append(m)

    from concourse.bass_utils import run_bass_kernel_spmd

    trace = bool(int(os.environ.get("KERNEL_TRACE", "0")))
    res = run_bass_kernel_spmd(
        nc, in_maps, core_ids=list(range(NCORES)), trace=trace
    )
    LAST_RESULTS = res

    # ---- assemble: concat vocab slices, reorder rows (t-major -> b-major) ----
    logits_tb = np.concatenate(
        [res.results[c]["out"] for c in range(NCORES)], axis=1
    )  # [T*B, V]
    logits = (
        logits_tb.reshape(T, B, V).transpose(1, 0, 2).reshape(B * T, V)
    )
    return np.ascontiguousarray(logits.astype(np.float32))
